# revision 16
# baseline (speedup 1.0000x reference)
"""FP8 semi-sparse activation linear kernel for Trainium2 (8 NeuronCores).

Computes: rowwise-fp8-quant(2:4-sparsify(relu(x)^2)) @ rowwise-fp8-quant(W).T -> bf16

Sharding: x rows split 4 ways (m-groups), W rows (= out cols) split 2 ways
(n-halves); core c handles m-group c % 4, n-half c // 4.

Key implementation notes:
  - TRN fp8e4 max is +-240 (vs OCP e4m3fn +-448).  We quantize to +-224
    (scale' = 2*scale_ref); powers of two commute with RNE so the fp8
    rounding grid matches the reference exactly (sans the denormal tail,
    which is ~2^-18 relative -- irrelevant).
  - The 2:4 sparsify runs on r = relu(x) (monotonic under squaring); the
    square is fused into the fp8 quantization via ACT's Square activation
    (out = Square(r * sqrt(inv))), which hardware evaluates exactly.
  - Transposes (K onto partitions) use the DMA xbar on bf16-bitcast fp8
    pairs: one dma_start_transpose per [128, 4096-fp8] tile.  Resulting
    layout: partition p, k-block b holds k = 256*b + 2*p + {0,1} as
    adjacent bytes.
  - The matmul runs in fp8 DoubleRow (2x) mode: the moving operand uses a
    [p, 2, n] AP (pair step 1 byte -- legal on the MM side), the
    stationary operand uses DoubleRowSwInterleave which expects adjacent
    A/B byte pairs with columns reversed; we pre-reverse x rows per
    128-tile on the host so PSUM rows come out in natural order.
  - Per-row x scales are computed on reversed rows; they are un-reversed
    on chip with a stream_shuffle (reverse within 32) + 4 partition-block
    DMA copies.
"""
import sys
import os
import dataclasses

sys.path.insert(0, "/opt/trn_rl_repo")

import numpy as np
import ml_dtypes

import concourse.bass as bass
import concourse.mybir as mybir
from concourse.tile import TileContext
from concourse.bass_utils import run_bass_kernel_spmd

# ---------------------------------------------------------------------------
# Workaround: this environment's walrus rejects instructions with more than
# a couple of sync-wait conditions ("Too many sync wait commands").  Split
# excess waits onto NoOp instructions inserted before the offender.
import orjson as _orjson

_orig_to_json_bytes = bass.Bass.to_json_bytes
_LIMIT_DEFAULT = 1
_ws_counter = [0]


def _split_waits(doc):
    for fn in doc.get("functions", []):
        for blk in fn.get("blocks", []):
            insts = blk.get("instructions")
            if not insts:
                continue
            out = []
            changed = False
            for ins in insts:
                si = ins.get("sync_info")
                if si:
                    waits = si.get("on_wait") or []
                    if len(waits) > _LIMIT_DEFAULT:
                        excess = waits[:-_LIMIT_DEFAULT]
                        keep = waits[-_LIMIT_DEFAULT:]
                        for i in range(0, len(excess), _LIMIT_DEFAULT):
                            _ws_counter[0] += 1
                            out.append({
                                "name": f"I-waitsplit-{_ws_counter[0]}",
                                "engine": ins["engine"],
                                "opcode": "NoOp",
                                "ins": [],
                                "outs": [],
                                "sync_info": {
                                    "on_wait": excess[i:i + _LIMIT_DEFAULT],
                                    "on_update": [],
                                },
                            })
                        si["on_wait"] = keep
                        changed = True
                out.append(ins)
            if changed:
                blk["instructions"] = out
    return doc


def _patched_to_json_bytes(self):
    return _orjson.dumps(_split_waits(_orjson.loads(_orig_to_json_bytes(self))))


bass.Bass.to_json_bytes = _patched_to_json_bytes
# ---------------------------------------------------------------------------

F32 = mybir.dt.float32
FP8 = mybir.dt.float8e4
BF16 = mybir.dt.bfloat16
ALU = mybir.AluOpType
ACTF = mybir.ActivationFunctionType

M, K, NW = 8192, 4096, 4096
N_CORES = 8
MG, NH = 4, 2                  # m-groups x n-halves
MS, NS = M // MG, NW // NH     # 2048 x 2048 per-core output shard
NT = MS // 128                 # 16 x-tiles
WT = NS // 128                 # 16 w-tiles
KB = K // 256                  # 16 k-blocks of 256
SQRT224 = float(np.float32(np.sqrt(np.float32(224.0))))
INV224 = float(np.float32(1.0) / np.float32(224.0))


def _build_program():
    nc = bass.Bass()
    xs = nc.dram_tensor("xs", [MS, K], F32, kind="ExternalInput")
    ws = nc.dram_tensor("ws", [NS, K], F32, kind="ExternalInput")
    out = nc.dram_tensor("out", [MS, NS], BF16, kind="ExternalOutput")
    wsd = nc.dram_tensor("wsd", [128, NS], F32, kind="Internal")

    rev32 = list(range(31, -1, -1))

    with TileContext(nc) as tc:
        with tc.tile_pool(name="persist", bufs=1) as cpool, \
             tc.tile_pool(name="work", bufs=1) as pool, \
             tc.tile_pool(name="psum", bufs=8, space="PSUM") as psp:

            WqT = cpool.tile([128, KB, NS], BF16)     # 8 MB
            WscaleB = cpool.tile([128, NS], F32)      # 1 MB
            wsrow = cpool.tile([1, NS], F32)

            # NOTE: every DMA (loads, xbar transposes, stores, small copies)
            # is issued from nc.sync -- concurrent DMA on another ring
            # corrupts in-flight xbar transposes (HW bug, verified).

            wtiles = {}

            def w_load(wt):
                wtile = pool.tile([128, K], F32, tag="wtile", bufs=3,
                                  name=f"wtile_{wt}")
                nc.sync.dma_start(out=wtile[:], in_=ws[wt * 128:(wt + 1) * 128])
                wtiles[wt] = wtile

            def w_rest(wt):
                wtile = wtiles.pop(wt)
                wabs = pool.tile([128, 1], F32, tag="sA", bufs=2, name=f"wabs_{wt}")
                nc.vector.tensor_reduce(out=wabs[:], in_=wtile[:],
                                        axis=mybir.AxisListType.X, op=ALU.max,
                                        apply_absolute_value=True)
                winv = pool.tile([128, 1], F32, tag="sB", bufs=2, name=f"winv_{wt}")
                nc.vector.reciprocal(out=winv[:], in_=wabs[:])
                winv2 = pool.tile([128, 1], F32, tag="sC", bufs=2, name=f"winv2_{wt}")
                nc.vector.tensor_scalar_mul(out=winv2[:], in0=winv[:], scalar1=224.0)
                wscale = pool.tile([128, 1], F32, tag="sD", bufs=2, name=f"wscale_{wt}")
                nc.vector.tensor_scalar_mul(out=wscale[:], in0=wabs[:], scalar1=INV224)
                nc.sync.dma_start(out=wsrow[0:1, wt * 128:(wt + 1) * 128],
                                  in_=wscale[:])
                wq = pool.tile([128, K], FP8, tag="wq8", bufs=1, name=f"wq_{wt}")
                nc.scalar.activation(out=wq[:], in_=wtile[:], func=ACTF.Copy,
                                     scale=winv2[:])
                nc.sync.dma_start_transpose(WqT[:, :, wt * 128:(wt + 1) * 128],
                                            wq[:].bitcast(BF16))

            xts = {}
            xqs = {}
            xqts = {}
            xshs = {}
            xnats = {}

            def x_pre(mt):
                xt = pool.tile([128, K], F32, tag="xt", bufs=2, name=f"xt_{mt}")
                nc.sync.dma_start(out=xt[:], in_=xs[mt * 128:(mt + 1) * 128])
                nc.scalar.activation(out=xt[:], in_=xt[:], func=ACTF.Relu)
                xts[mt] = xt

            def x_post(mt):
                r = xts[mt]
                r2 = r[:].rearrange("p (g two) -> p g two", two=2)
                pr = pool.tile([128, K // 2], F32, tag="pr", name=f"pr_{mt}")
                qs = pool.tile([128, K // 2], F32, tag="qs", name=f"qs_{mt}")
                nc.vector.tensor_tensor(out=pr[:], in0=r2[:, :, 0], in1=r2[:, :, 1], op=ALU.max)
                nc.vector.tensor_tensor(out=qs[:], in0=r2[:, :, 0], in1=r2[:, :, 1], op=ALU.min)

                rmax = pool.tile([128, 1], F32, tag="sE", bufs=2, name=f"rmax_{mt}")
                nc.vector.tensor_reduce(out=rmax[:], in_=pr[:],
                                        axis=mybir.AxisListType.X, op=ALU.max)

                pr2 = pr[:].rearrange("p (g two) -> p g two", two=2)
                qs2 = qs[:].rearrange("p (g two) -> p g two", two=2)
                u1t = pool.tile([128, K // 4], F32, tag="u1", name=f"u1_{mt}")
                u2t = pool.tile([128, K // 4], F32, tag="pr", name=f"u2_{mt}")
                nc.vector.tensor_tensor(out=u1t[:], in0=pr2[:, :, 0], in1=pr2[:, :, 1], op=ALU.min)
                nc.vector.tensor_tensor(out=u2t[:], in0=qs2[:, :, 0], in1=qs2[:, :, 1], op=ALU.max)
                # same-position in-place max (same pattern as the mask mult)
                nc.vector.tensor_tensor(out=u1t[:], in0=u1t[:], in1=u2t[:], op=ALU.max)
                thr = u1t[:]

                mask = pool.tile([128, K], FP8, tag="qs", name=f"mask_{mt}")
                r4 = r[:].rearrange("p (g four) -> p g four", four=4)
                m4 = mask[:].rearrange("p (g four) -> p g four", four=4)
                tb = thr.rearrange("p (g one) -> p g one", one=1)
                tb = dataclasses.replace(tb, ap=[tb.ap[0], tb.ap[1], [0, 4]])
                nc.vector.tensor_tensor(out=m4[:], in0=r4[:], in1=tb, op=ALU.is_ge)
                nc.vector.tensor_tensor(out=r[:], in0=r[:], in1=mask[:], op=ALU.mult)

                rm2 = pool.tile([128, 1], F32, tag="sF", bufs=2, name=f"rm2_{mt}")
                nc.vector.tensor_scalar_max(out=rm2[:], in0=rmax[:], scalar1=1e-5)
                rrec = pool.tile([128, 1], F32, tag="sG", bufs=2, name=f"rrec_{mt}")
                nc.vector.reciprocal(out=rrec[:], in_=rm2[:])
                sq = pool.tile([128, 1], F32, tag="sH", bufs=2, name=f"sq_{mt}")
                nc.vector.tensor_scalar_mul(out=sq[:], in0=rrec[:], scalar1=SQRT224)
                xsc = pool.tile([128, 1], F32, tag="sI", bufs=2, name=f"xsc_{mt}")
                nc.vector.tensor_tensor(out=xsc[:], in0=rmax[:], in1=rmax[:], op=ALU.mult)
                xsc2 = pool.tile([128, 1], F32, tag="sJ", bufs=2, name=f"xsc2_{mt}")
                nc.vector.tensor_scalar_mul(out=xsc2[:], in0=xsc[:], scalar1=INV224)
                xsh = pool.tile([128, 1], F32, tag="sK", bufs=2, name=f"xsh_{mt}")
                nc.vector.stream_shuffle(out=xsh[:], in_=xsc2[:], mask=rev32)
                xshs[mt] = xsh

                xq = pool.tile([128, K], FP8, tag="xq8", bufs=2, name=f"xq_{mt}")
                nc.scalar.activation(out=xq[:], in_=r[:], func=ACTF.Square, scale=sq[:])
                xqs[mt] = xq

            def mm_mm(mt):
                xq = xqs[mt]
                xqT = pool.tile([128, KB, 128], BF16, tag="xqT", bufs=3,
                                name=f"xqT_{mt}")
                nc.sync.dma_start_transpose(xqT[:], xq[:].bitcast(BF16))
                xqts[mt] = xqT
                xnat = pool.tile([128, 1], F32, tag="sL", bufs=2, name=f"xnat_{mt}")
                xsh = xshs[mt]
                for q in range(4):
                    nc.sync.dma_start(out=xnat[32 * (3 - q):32 * (4 - q)],
                                      in_=xsh[32 * q:32 * (q + 1)])
                xnats[mt] = xnat
                accs = [psp.tile([128, 512], F32, tag="acc", name=f"acc_{mt}_{ch}")
                        for ch in range(4)]
                wq8 = WqT[:].bitcast(FP8)  # [128, KB, 2*NS]
                xq8 = xqT[:].bitcast(FP8)  # [128, KB, 256]
                for ch in range(4):
                    for blk in range(KB):
                        lhs = xq8[:, blk, :]
                        rhs = wq8[:, blk, ch * 1024:(ch + 1) * 1024].rearrange(
                            "p (n two) -> p two n", two=2)
                        nc.tensor.matmul(accs[ch][:], lhs, rhs,
                                         start=(blk == 0), stop=(blk == KB - 1),
                                         perf_mode=mybir.MatmulPerfMode.DoubleRowSwInterleave)
                return accs

            def dequant(mt, accs):
                xnat = xnats[mt]
                ost = pool.tile([128, NS], BF16, tag="wq8", bufs=1, name=f"ost_{mt}")
                for ch in range(4):
                    nc.vector.scalar_tensor_tensor(
                        out=ost[:, ch * 512:(ch + 1) * 512],
                        in0=accs[ch][:], scalar=xnat[:],
                        in1=WscaleB[:, ch * 512:(ch + 1) * 512],
                        op0=ALU.mult, op1=ALU.mult)
                nc.sync.dma_start(out=out[mt * 128:(mt + 1) * 128], in_=ost[:])

            # ---- emission schedule (single DMA ring, software-pipelined) ----
            def bcast_all():
                # replicate wsrow [1, NS] across 128 partitions via a DRAM
                # doubling chain (PE-free, ring-only)
                nc.sync.dma_start(out=wsd[0:1], in_=wsrow[:])
                k = 1
                while k < 128:
                    nc.sync.dma_start(out=wsd[k:2 * k], in_=wsd[0:k])
                    k *= 2
                nc.sync.dma_start(out=WscaleB[:], in_=wsd[:])

            # W pipeline: loads run 2 ahead of the compute+transpose tail so the
            # single DMA ring never stalls a load behind a compute-gated
            # transpose.  x tiles 0/1 load early to warm the X pipeline.
            w_load(0)
            w_load(1)
            x_pre(0)
            x_pre(1)
            for wt in range(2, WT):
                w_load(wt)
                w_rest(wt - 2)
            w_rest(WT - 2)
            w_rest(WT - 1)
            bcast_all()
            x_post(0)

            pending = {}
            for mt in range(NT):
                if mt >= 1:
                    x_post(mt)
                pending[mt] = mm_mm(mt)
                if mt + 2 < NT:
                    x_pre(mt + 2)
                if mt >= 1:
                    dequant(mt - 1, pending.pop(mt - 1))
            dequant(NT - 1, pending.pop(NT - 1))

    return nc


_cached_nc = None


def _get_nc():
    global _cached_nc
    if _cached_nc is None:
        _cached_nc = _build_program()
    return _cached_nc


def _run(x, W, trace=False):
    x = np.ascontiguousarray(x, dtype=np.float32)
    W = np.ascontiguousarray(W, dtype=np.float32)
    assert x.shape == (M, K) and W.shape == (NW, K)
    nc = _get_nc()
    in_maps = []
    for c in range(N_CORES):
        g, h = c % MG, c // MG
        xsh = x[g * MS:(g + 1) * MS].reshape(NT, 128, K)[:, ::-1, :].reshape(MS, K)
        in_maps.append({
            "xs": np.ascontiguousarray(xsh),
            "ws": W[h * NS:(h + 1) * NS],
        })
    res = run_bass_kernel_spmd(nc, in_maps, core_ids=list(range(N_CORES)),
                               trace=trace)
    outf = np.empty((M, NW), dtype=ml_dtypes.bfloat16)
    for c in range(N_CORES):
        g, h = c % MG, c // MG
        outf[g * MS:(g + 1) * MS, h * NS:(h + 1) * NS] = res.results[c]["out"]
    return outf, res


def kernel(x, W):
    out, _ = _run(x, W, trace=False)
    return out


# revision 18
# speedup vs baseline: 1.0124x; 1.0124x over previous
"""FP8 semi-sparse activation linear kernel for Trainium2 (8 NeuronCores).

Computes: rowwise-fp8-quant(2:4-sparsify(relu(x)^2)) @ rowwise-fp8-quant(W).T -> bf16

Sharding: x rows split 4 ways (m-groups), W rows (= out cols) split 2 ways
(n-halves); core c handles m-group c % 4, n-half c // 4.

Key implementation notes:
  - TRN fp8e4 max is +-240 (vs OCP e4m3fn +-448).  We quantize to +-224
    (scale' = 2*scale_ref); powers of two commute with RNE so the fp8
    rounding grid matches the reference exactly (sans the denormal tail,
    which is ~2^-18 relative -- irrelevant).
  - The 2:4 sparsify runs on r = relu(x) (monotonic under squaring); the
    square is fused into the fp8 quantization via ACT's Square activation
    (out = Square(r * sqrt(inv))), which hardware evaluates exactly.
  - Transposes (K onto partitions) use the DMA xbar on bf16-bitcast fp8
    pairs: one dma_start_transpose per [128, 4096-fp8] tile.  Resulting
    layout: partition p, k-block b holds k = 256*b + 2*p + {0,1} as
    adjacent bytes.
  - The matmul runs in fp8 DoubleRow (2x) mode: the moving operand uses a
    [p, 2, n] AP (pair step 1 byte -- legal on the MM side), the
    stationary operand uses DoubleRowSwInterleave which expects adjacent
    A/B byte pairs with columns reversed; we pre-reverse x rows per
    128-tile on the host so PSUM rows come out in natural order.
  - Per-row x scales are computed on reversed rows; they are un-reversed
    on chip with a stream_shuffle (reverse within 32) + 4 partition-block
    DMA copies.
"""
import sys
import os
import dataclasses

sys.path.insert(0, "/opt/trn_rl_repo")

import numpy as np
import ml_dtypes

import concourse.bass as bass
import concourse.mybir as mybir
from concourse.tile import TileContext
from concourse.bass_utils import run_bass_kernel_spmd

# ---------------------------------------------------------------------------
# Workaround: this environment's walrus rejects instructions with more than
# a couple of sync-wait conditions ("Too many sync wait commands").  Split
# excess waits onto NoOp instructions inserted before the offender.
import orjson as _orjson

_orig_to_json_bytes = bass.Bass.to_json_bytes
_LIMIT_DEFAULT = 1
_ws_counter = [0]


def _split_waits(doc):
    for fn in doc.get("functions", []):
        for blk in fn.get("blocks", []):
            insts = blk.get("instructions")
            if not insts:
                continue
            out = []
            changed = False
            for ins in insts:
                si = ins.get("sync_info")
                if si:
                    waits = si.get("on_wait") or []
                    if len(waits) > _LIMIT_DEFAULT:
                        excess = waits[:-_LIMIT_DEFAULT]
                        keep = waits[-_LIMIT_DEFAULT:]
                        for i in range(0, len(excess), _LIMIT_DEFAULT):
                            _ws_counter[0] += 1
                            out.append({
                                "name": f"I-waitsplit-{_ws_counter[0]}",
                                "engine": ins["engine"],
                                "opcode": "NoOp",
                                "ins": [],
                                "outs": [],
                                "sync_info": {
                                    "on_wait": excess[i:i + _LIMIT_DEFAULT],
                                    "on_update": [],
                                },
                            })
                        si["on_wait"] = keep
                        changed = True
                out.append(ins)
            if changed:
                blk["instructions"] = out
    return doc


def _patched_to_json_bytes(self):
    return _orjson.dumps(_split_waits(_orjson.loads(_orig_to_json_bytes(self))))


bass.Bass.to_json_bytes = _patched_to_json_bytes
# ---------------------------------------------------------------------------

F32 = mybir.dt.float32
FP8 = mybir.dt.float8e4
BF16 = mybir.dt.bfloat16
ALU = mybir.AluOpType
ACTF = mybir.ActivationFunctionType

M, K, NW = 8192, 4096, 4096
N_CORES = 8
MG, NH = 4, 2                  # m-groups x n-halves
MS, NS = M // MG, NW // NH     # 2048 x 2048 per-core output shard
NT = MS // 128                 # 16 x-tiles
WT = NS // 128                 # 16 w-tiles
KB = K // 256                  # 16 k-blocks of 256
SQRT224 = float(np.float32(np.sqrt(np.float32(224.0))))
INV224 = float(np.float32(1.0) / np.float32(224.0))


def _build_program():
    nc = bass.Bass()
    xs = nc.dram_tensor("xs", [MS, K], F32, kind="ExternalInput")
    ws = nc.dram_tensor("ws", [NS, K], F32, kind="ExternalInput")
    out = nc.dram_tensor("out", [MS, NS], BF16, kind="ExternalOutput")
    wsd = nc.dram_tensor("wsd", [128, NS], F32, kind="Internal")

    rev32 = list(range(31, -1, -1))

    with TileContext(nc) as tc:
        with tc.tile_pool(name="persist", bufs=1) as cpool, \
             tc.tile_pool(name="work", bufs=1) as pool, \
             tc.tile_pool(name="psum", bufs=8, space="PSUM") as psp:

            WqT = cpool.tile([128, KB, NS], BF16)     # 8 MB
            WscaleB = cpool.tile([128, NS], F32)      # 1 MB
            wsrow = WscaleB  # row 0 doubles as the scale gather row

            # NOTE: every DMA (loads, xbar transposes, stores, small copies)
            # is issued from nc.sync -- concurrent DMA on another ring
            # corrupts in-flight xbar transposes (HW bug, verified).

            wtiles = {}

            def w_load(wt):
                wtile = pool.tile([128, K], F32, tag="wtile", bufs=3,
                                  name=f"wtile_{wt}")
                nc.sync.dma_start(out=wtile[:], in_=ws[wt * 128:(wt + 1) * 128])
                wtiles[wt] = wtile

            def w_rest(wt):
                wtile = wtiles.pop(wt)
                wabs = pool.tile([128, 1], F32, tag="sA", bufs=2, name=f"wabs_{wt}")
                nc.vector.tensor_reduce(out=wabs[:], in_=wtile[:],
                                        axis=mybir.AxisListType.X, op=ALU.max,
                                        apply_absolute_value=True)
                winv = pool.tile([128, 1], F32, tag="sB", bufs=2, name=f"winv_{wt}")
                nc.vector.reciprocal(out=winv[:], in_=wabs[:])
                winv2 = pool.tile([128, 1], F32, tag="sC", bufs=2, name=f"winv2_{wt}")
                nc.vector.tensor_scalar_mul(out=winv2[:], in0=winv[:], scalar1=224.0)
                wscale = pool.tile([128, 1], F32, tag="sD", bufs=2, name=f"wscale_{wt}")
                nc.vector.tensor_scalar_mul(out=wscale[:], in0=wabs[:], scalar1=INV224)
                nc.sync.dma_start(out=wsrow[0:1, wt * 128:(wt + 1) * 128],
                                  in_=wscale[:])
                wq = pool.tile([128, K], FP8, tag="wq8", bufs=1, name=f"wq_{wt}")
                nc.scalar.activation(out=wq[:], in_=wtile[:], func=ACTF.Copy,
                                     scale=winv2[:])
                nc.sync.dma_start_transpose(WqT[:, :, wt * 128:(wt + 1) * 128],
                                            wq[:].bitcast(BF16))

            xts = {}
            xqs = {}
            xqts = {}
            xshs = {}
            xnats = {}

            def x_pre(mt):
                xt = pool.tile([128, K], F32, tag="xt", bufs=2, name=f"xt_{mt}")
                nc.sync.dma_start(out=xt[:], in_=xs[mt * 128:(mt + 1) * 128])
                nc.scalar.activation(out=xt[:], in_=xt[:], func=ACTF.Relu)
                xts[mt] = xt

            def x_post(mt):
                r = xts[mt]
                r2 = r[:].rearrange("p (g two) -> p g two", two=2)
                pr = pool.tile([128, K // 2], F32, tag="pr", name=f"pr_{mt}")
                qs = pool.tile([128, K // 2], F32, tag="qs", name=f"qs_{mt}")
                nc.vector.tensor_tensor(out=pr[:], in0=r2[:, :, 0], in1=r2[:, :, 1], op=ALU.max)
                nc.vector.tensor_tensor(out=qs[:], in0=r2[:, :, 0], in1=r2[:, :, 1], op=ALU.min)

                rmax = pool.tile([128, 1], F32, tag="sE", bufs=2, name=f"rmax_{mt}")
                nc.vector.tensor_reduce(out=rmax[:], in_=pr[:],
                                        axis=mybir.AxisListType.X, op=ALU.max)

                pr2 = pr[:].rearrange("p (g two) -> p g two", two=2)
                qs2 = qs[:].rearrange("p (g two) -> p g two", two=2)
                u1t = pool.tile([128, K // 4], F32, tag="u1", name=f"u1_{mt}")
                u2t = pool.tile([128, K // 4], F32, tag="pr", name=f"u2_{mt}")
                nc.vector.tensor_tensor(out=u1t[:], in0=pr2[:, :, 0], in1=pr2[:, :, 1], op=ALU.min)
                nc.vector.tensor_tensor(out=u2t[:], in0=qs2[:, :, 0], in1=qs2[:, :, 1], op=ALU.max)
                # same-position in-place max (same pattern as the mask mult)
                nc.vector.tensor_tensor(out=u1t[:], in0=u1t[:], in1=u2t[:], op=ALU.max)
                thr = u1t[:]

                mask = pool.tile([128, K], FP8, tag="qs", name=f"mask_{mt}")
                r4 = r[:].rearrange("p (g four) -> p g four", four=4)
                m4 = mask[:].rearrange("p (g four) -> p g four", four=4)
                tb = thr.rearrange("p (g one) -> p g one", one=1)
                tb = dataclasses.replace(tb, ap=[tb.ap[0], tb.ap[1], [0, 4]])
                nc.vector.tensor_tensor(out=m4[:], in0=r4[:], in1=tb, op=ALU.is_ge)
                nc.vector.tensor_tensor(out=r[:], in0=r[:], in1=mask[:], op=ALU.mult)

                rm2 = pool.tile([128, 1], F32, tag="sF", bufs=2, name=f"rm2_{mt}")
                nc.vector.tensor_scalar_max(out=rm2[:], in0=rmax[:], scalar1=1e-5)
                rrec = pool.tile([128, 1], F32, tag="sG", bufs=2, name=f"rrec_{mt}")
                nc.vector.reciprocal(out=rrec[:], in_=rm2[:])
                sq = pool.tile([128, 1], F32, tag="sH", bufs=2, name=f"sq_{mt}")
                nc.vector.tensor_scalar_mul(out=sq[:], in0=rrec[:], scalar1=SQRT224)
                xsc = pool.tile([128, 1], F32, tag="sI", bufs=2, name=f"xsc_{mt}")
                nc.vector.tensor_tensor(out=xsc[:], in0=rmax[:], in1=rmax[:], op=ALU.mult)
                xsc2 = pool.tile([128, 1], F32, tag="sJ", bufs=2, name=f"xsc2_{mt}")
                nc.vector.tensor_scalar_mul(out=xsc2[:], in0=xsc[:], scalar1=INV224)
                xsh = pool.tile([128, 1], F32, tag="sK", bufs=2, name=f"xsh_{mt}")
                nc.vector.stream_shuffle(out=xsh[:], in_=xsc2[:], mask=rev32)
                xshs[mt] = xsh

                xq = pool.tile([128, K], FP8, tag="xq8", bufs=2, name=f"xq_{mt}")
                nc.scalar.activation(out=xq[:], in_=r[:], func=ACTF.Square, scale=sq[:])
                xqs[mt] = xq

            def mm_mm(mt):
                xq = xqs[mt]
                xqT = pool.tile([128, KB, 128], BF16, tag="xqT", bufs=4,
                                name=f"xqT_{mt}")
                nc.sync.dma_start_transpose(xqT[:], xq[:].bitcast(BF16))
                xqts[mt] = xqT
                xnat = pool.tile([128, 1], F32, tag="sL", bufs=2, name=f"xnat_{mt}")
                xsh = xshs[mt]
                for q in range(4):
                    nc.sync.dma_start(out=xnat[32 * (3 - q):32 * (4 - q)],
                                      in_=xsh[32 * q:32 * (q + 1)])
                xnats[mt] = xnat
                accs = [psp.tile([128, 512], F32, tag="acc", name=f"acc_{mt}_{ch}")
                        for ch in range(4)]
                wq8 = WqT[:].bitcast(FP8)  # [128, KB, 2*NS]
                xq8 = xqT[:].bitcast(FP8)  # [128, KB, 256]
                for ch in range(4):
                    for blk in range(KB):
                        lhs = xq8[:, blk, :]
                        rhs = wq8[:, blk, ch * 1024:(ch + 1) * 1024].rearrange(
                            "p (n two) -> p two n", two=2)
                        nc.tensor.matmul(accs[ch][:], lhs, rhs,
                                         start=(blk == 0), stop=(blk == KB - 1),
                                         perf_mode=mybir.MatmulPerfMode.DoubleRowSwInterleave)
                return accs

            def dequant(mt, accs):
                xnat = xnats[mt]
                ost = pool.tile([128, NS], BF16, tag="wq8", bufs=1, name=f"ost_{mt}")
                for ch in range(4):
                    nc.vector.scalar_tensor_tensor(
                        out=ost[:, ch * 512:(ch + 1) * 512],
                        in0=accs[ch][:], scalar=xnat[:],
                        in1=WscaleB[:, ch * 512:(ch + 1) * 512],
                        op0=ALU.mult, op1=ALU.mult)
                nc.sync.dma_start(out=out[mt * 128:(mt + 1) * 128], in_=ost[:])

            # ---- emission schedule (single DMA ring, software-pipelined) ----
            def bcast_all():
                # replicate wsrow [1, NS] across 128 partitions via a DRAM
                # doubling chain (PE-free, ring-only)
                nc.sync.dma_start(out=wsd[0:1], in_=wsrow[0:1, :])
                k = 1
                while k < 128:
                    nc.sync.dma_start(out=wsd[k:2 * k], in_=wsd[0:k])
                    k *= 2
                nc.sync.dma_start(out=WscaleB[:], in_=wsd[:])

            # W pipeline: loads run 2 ahead of the compute+transpose tail so the
            # single DMA ring never stalls a load behind a compute-gated
            # transpose.  x tiles 0/1 load early to warm the X pipeline.
            w_load(0)
            w_load(1)
            x_pre(0)
            x_pre(1)
            for wt in range(2, WT):
                w_load(wt)
                w_rest(wt - 2)
            w_rest(WT - 2)
            w_rest(WT - 1)
            bcast_all()
            x_post(0)

            pending = {}
            for mt in range(NT):
                if mt >= 1:
                    x_post(mt)
                pending[mt] = mm_mm(mt)
                if mt + 2 < NT:
                    x_pre(mt + 2)
                if mt >= 1:
                    dequant(mt - 1, pending.pop(mt - 1))
            dequant(NT - 1, pending.pop(NT - 1))

    return nc


_cached_nc = None


def _get_nc():
    global _cached_nc
    if _cached_nc is None:
        _cached_nc = _build_program()
    return _cached_nc


def _run(x, W, trace=False):
    x = np.ascontiguousarray(x, dtype=np.float32)
    W = np.ascontiguousarray(W, dtype=np.float32)
    assert x.shape == (M, K) and W.shape == (NW, K)
    nc = _get_nc()
    in_maps = []
    for c in range(N_CORES):
        g, h = c % MG, c // MG
        xsh = x[g * MS:(g + 1) * MS].reshape(NT, 128, K)[:, ::-1, :].reshape(MS, K)
        in_maps.append({
            "xs": np.ascontiguousarray(xsh),
            "ws": W[h * NS:(h + 1) * NS],
        })
    res = run_bass_kernel_spmd(nc, in_maps, core_ids=list(range(N_CORES)),
                               trace=trace)
    outf = np.empty((M, NW), dtype=ml_dtypes.bfloat16)
    for c in range(N_CORES):
        g, h = c % MG, c // MG
        outf[g * MS:(g + 1) * MS, h * NS:(h + 1) * NS] = res.results[c]["out"]
    return outf, res


def kernel(x, W):
    out, _ = _run(x, W, trace=False)
    return out
